# revision 27
# baseline (speedup 1.0000x reference)
"""ArcFace (AngularPenaltySMLoss) distributed Trainium2 kernel.

Strategy (tensor-parallel over classes, per the sharding hint):
  - Shard W's C=100000 rows over 8 cores (12500 each, zero-padded to 12544;
    the host subtracts the pad's deterministic exp(0) contribution).
  - Host: normalize x, pre-scale + cast both operands to fp8e4m3, and
    REPACK them chunk-major so every DMA line is 1-4KB contiguous per SBUF
    partition: the HWDGE queue sustains ~60GB/s on 1KB scattered lines
    (the previous bottleneck -- 6.5MB of W per core paced the kernel) and
    much more on contiguous 4KB lines. All wt rides the Sync queue; the
    Activation queue's DGE contends with the busy ACT engine, so it only
    carries the late xn.T slices early in the run.
  - Device (SPMD, no collectives): logits tile = xnT.T @ WT chunk into PSUM
    via DoubleRow fp8 matmuls. The per-sample sum of exp(s*logit) over the
    local classes is split ~64:36 between two engines so neither paces the
    PE (ACT alone measures ~92us of demand vs the PE's ~86us stream):
      * ACT: exp with fused free-dim accumulate (accum_out).
      * DVE: Schraudolph bit-trick exp (int32(z*A + B) bitcast to f32,
        calibrated to zero mean bias over the logit distribution) + a
        bitcast reduce_sum. ~0.2% per-tile sum error on ~36% of classes
        -> ~1e-5 on the final loss.
    Partial sums land in per-engine planes of one accumulator tile, DMA'd
    out (all-but-last chunk early) and combined on host.
  - Host: sum partials over cores/slots, subtract the pad term, compute the
    (tiny) per-sample target / arccos / log path in f64, return the loss.
"""

import sys

if "/opt/trn_rl_repo" not in sys.path:
    sys.path.insert(0, "/opt/trn_rl_repo")

import ml_dtypes
import numpy as np

import concourse.bass as bass
import concourse.mybir as mybir
from concourse import bacc
from concourse.bass_utils import run_bass_kernel_spmd
from concourse.tile import TileContext

B, C, D = 1024, 100000, 512
S_SCALE, MARGIN, EPS = 64.0, 0.5, 1e-7
N_CORES = 8
C_SHARD = C // N_CORES          # 12500
P = 128
KO = D // P                     # 4 k-chunks of 128
B_TILES = B // P                # 8
CHUNK = 1024                    # classes per PSUM tile (2 banks; 4 tiles ring)
TAIL = 256                      # last (padded) chunk width
C_PAD = 12 * CHUNK + TAIL       # 12544: C_SHARD padded with 44 zero classes
MM_N = 512                      # one matmul output <= one PSUM bank
N_WARM = 5                      # PE warm-up matmuls (HAM runway over the fill)
DVE_MOD = 14                    # tile t -> DVE when t % DVE_MOD in DVE_RES
DVE_RES = (2, 5, 8, 11, 13)     # f=0.357: ACT and DVE both ~82us, under the PE

# fp8e4m3 with pre-scaling to dodge subnormals; exp scale folds it back out.
WSCALE, XSCALE = 8.0, 4.0
NPDT = ml_dtypes.float8_e4m3
MDT = mybir.dt.float8e4
ACT_SCALE = S_SCALE / (WSCALE * XSCALE)   # exp(ACT_SCALE * psum) = exp(s*logit)

# Schraudolph exp in PSUM units: exp(ACT_SCALE*v) ~= bitcast_f32(int32(A*v+B)).
# C_CAL calibrated to zero the mean bias of sum-exp over z ~ N(0, 1.633^2)
# (the s*logit marginal for these inputs).
LOG2E = 1.4426950408889634
C_CAL = 483053.0
TS_A = ACT_SCALE * LOG2E * (1 << 23)
TS_B = 127.0 * (1 << 23) - C_CAL
# what the DVE path yields for a zero logit (pad classes): bitcast(int32(B))
PAD_EXP_DVE = float(
    np.int32(round(float(np.float32(TS_B)))).view(np.float32)
)


def _chunks():
    spans = []
    c0 = 0
    while c0 + CHUNK <= C_PAD - TAIL:
        spans.append((c0, CHUNK))
        c0 += CHUNK
    spans.append((c0, TAIL))
    return spans


def _is_dve(t):
    return t % DVE_MOD in DVE_RES


LAST_RESULT = None
_NC_CACHE = None


def _build_bass():
    spans = _chunks()
    n_chunks = len(spans)

    nc = bacc.Bacc("TRN2")
    # both inputs repacked on host: per-partition lines are contiguous
    # [bt][ko][128] / [ci][ko][cw] blocks
    xnt = nc.declare_dram_parameter("xnt", [P, KO * B], MDT, isOutput=False)
    wt = nc.declare_dram_parameter("wt", [P, KO * C_PAD], MDT, isOutput=False)
    out = nc.declare_dram_parameter(
        "out", [P, n_chunks, 2, B_TILES], mybir.dt.float32, isOutput=True
    )

    with TileContext(nc) as tc:
        with (
            tc.tile_pool(name="xpool", bufs=1) as xpool,
            tc.tile_pool(name="wpool", bufs=4) as wpool,
            tc.tile_pool(name="ipool", bufs=4) as ipool,
            tc.tile_pool(name="accp", bufs=1) as accp,
            tc.tile_pool(name="psum", bufs=4, space="PSUM") as psum,
        ):
            # xn.T resident in SBUF: [p, bt, ko, 128], row d = ko*128 + p.
            # 8 per-b-tile DMAs (512B contiguous lines) so the first matmul
            # only waits for slice 0; the slices needed latest ride the
            # Activation queue before ACT gets busy.
            xnt_sb = xpool.tile([P, B_TILES, KO, P], MDT)

            def xnt_slice_dma(eng, bt):
                eng.dma_start(
                    xnt_sb[:, bt],
                    xnt[:, bt * KO * P : (bt + 1) * KO * P].rearrange(
                        "p (ko b) -> p ko b", ko=KO
                    ),
                )

            # per-(chunk, b-tile) partial sums of exp(s * logit); plane 0 is
            # written by ACT accum, plane 1 by the DVE reduce; chunk-major so
            # the early/final output DMAs slice contiguously. memset so
            # unowned slots stay zero and the host just sums everything.
            acc = accp.tile([P, n_chunks, 2, B_TILES], mybir.dt.float32)
            nc.vector.memset(acc[:], 0)

            # PE warm-up: HAM un-throttles (1.2 -> 2.4 GHz) only after
            # ~3us of sustained matmul activity; these bridge the PE from
            # engine-start to the first data-dependent matmul.
            wsrc = xpool.tile([P, MM_N], MDT, tag="warm_src")
            nc.vector.memset(wsrc[:], 1)
            for _ in range(N_WARM):
                pw = psum.tile([P, CHUNK], mybir.dt.float32, tag="ps")
                nc.tensor.matmul(
                    pw[:, :MM_N], wsrc[:, :P], wsrc[:], start=True, stop=True
                )

            for ci, (c0, cw) in enumerate(spans):
                tag = "wt" if cw == CHUNK else "wt_tail"
                wt_tile = wpool.tile([P, KO, cw], MDT, tag=tag)
                off = KO * c0
                src = wt[:, off : off + KO * cw].rearrange(
                    "p (ko c) -> p ko c", ko=KO
                )
                if ci == 0:
                    # startup choreography on Sync: slice bt0, then the
                    # first chunk in two halves (the first matmuls only
                    # gate on bt0 + ko01), then nearby slices; late
                    # b-tiles ride the still-idle Activation queue.
                    xnt_slice_dma(nc.sync, 0)
                    nc.sync.dma_start(wt_tile[:, :2], src[:, :2])
                    xnt_slice_dma(nc.sync, 1)
                    nc.sync.dma_start(wt_tile[:, 2:], src[:, 2:])
                    xnt_slice_dma(nc.sync, 2)
                    xnt_slice_dma(nc.sync, 3)
                    for bt in range(4, B_TILES):
                        xnt_slice_dma(nc.scalar, bt)
                else:
                    # one contiguous 1-4KB-per-partition transfer on Sync
                    nc.sync.dma_start(wt_tile[:], src[:])
                if ci == n_chunks - 1:
                    # ship all but the final chunk's slots early (behind the
                    # last wt chunk in the queue) so the closing DMA + drain
                    # tail is short
                    nc.sync.dma_start(
                        out[:, : n_chunks - 1], acc[:, : n_chunks - 1]
                    )

                for bt in range(B_TILES):
                    ps = psum.tile([P, CHUNK], mybir.dt.float32, tag="ps")
                    n_sub = (cw + MM_N - 1) // MM_N
                    for si in range(n_sub):
                        s0 = si * MM_N
                        sw = min(MM_N, cw - s0)
                        for k in range(0, KO, 2):
                            nc.tensor.matmul(
                                ps[:, s0 : s0 + sw],
                                xnt_sb[:, bt, k : k + 2, :],
                                wt_tile[:, k : k + 2, s0 : s0 + sw],
                                start=(k == 0),
                                stop=(k + 2 >= KO),
                                perf_mode=mybir.MatmulPerfMode.DoubleRow,
                            )
                    t = ci * B_TILES + bt
                    if _is_dve(t):
                        # DVE: Schraudolph exp + bitcast reduce
                        it = ipool.tile([P, CHUNK], mybir.dt.int32, tag="i32")
                        nc.vector.tensor_scalar(
                            it[:, :cw],
                            ps[:, :cw],
                            TS_A,
                            TS_B,
                            mybir.AluOpType.mult,
                            mybir.AluOpType.add,
                        )
                        nc.vector.reduce_sum(
                            acc[:, ci, 1, bt : bt + 1],
                            it[:, :cw].bitcast(mybir.dt.float32),
                            axis=mybir.AxisListType.X,
                        )
                    else:
                        # ACT: exp elementwise (in place) + fused accumulate
                        nc.scalar.activation(
                            ps[:, :cw],
                            ps[:, :cw],
                            mybir.ActivationFunctionType.Exp,
                            scale=ACT_SCALE,
                            accum_out=acc[:, ci, 0, bt : bt + 1],
                        )

            nc.sync.dma_start(out[:, n_chunks - 1 :], acc[:, n_chunks - 1 :])

    nc.compile()
    return nc


def _get_nc():
    global _NC_CACHE
    if _NC_CACHE is None:
        _NC_CACHE = _build_bass()
    return _NC_CACHE


def _pack_wt(shard_t_q: np.ndarray) -> np.ndarray:
    """[D, C_PAD] fp8 -> [P, KO*C_PAD], per-partition [ci][ko][cw] blocks."""
    parts = []
    for c0, cw in _chunks():
        blk = shard_t_q[:, c0 : c0 + cw].reshape(KO, P, cw)
        parts.append(blk.transpose(1, 0, 2).reshape(P, KO * cw))
    return np.ascontiguousarray(np.concatenate(parts, axis=1))


def kernel(x: np.ndarray, labels: np.ndarray, W: np.ndarray) -> np.ndarray:
    global LAST_RESULT
    x = np.asarray(x, dtype=np.float32)
    W = np.asarray(W, dtype=np.float32)
    labels = np.asarray(labels)

    # ---- host prep (sharding glue) ----
    norms = np.maximum(np.sqrt((x.astype(np.float64) ** 2).sum(axis=1)), 1e-12)
    xn = (x / norms[:, None].astype(np.float32)).astype(np.float32)
    xnt_q = np.ascontiguousarray(xn.T * XSCALE).astype(NPDT)  # [D, B]
    # repack to [P, bt][ko][128] contiguous lines
    xnt_lin = np.ascontiguousarray(
        xnt_q.reshape(KO, P, B_TILES, P).transpose(1, 2, 0, 3).reshape(P, KO * B)
    )

    n_pad = C_PAD - C_SHARD
    in_maps = []
    for i in range(N_CORES):
        shard = W[i * C_SHARD : (i + 1) * C_SHARD]
        wt_q = (shard.T * WSCALE).astype(NPDT)                # [D, C_SHARD]
        wt_q = np.concatenate(
            [wt_q, np.zeros((D, n_pad), dtype=NPDT)], axis=1
        )
        in_maps.append({"xnt": xnt_lin, "wt": _pack_wt(wt_q)})

    # ---- device: per-core partial sum over classes of exp(s*logit) ----
    nc = _get_nc()
    res = run_bass_kernel_spmd(nc, in_maps, core_ids=list(range(N_CORES)))
    LAST_RESULT = res

    # ---- host combine (the all-reduce + tiny per-sample tail) ----
    n_chunks = len(_chunks())
    sumexp = np.zeros(B, dtype=np.float64)
    for i in range(N_CORES):
        part = res.results[i]["out"].astype(np.float64)  # [P, NC, 2, BT]
        sumexp += part.sum(axis=(1, 2)).T.reshape(B)     # b = bt*128 + p
    # subtract the pad classes' contribution (exact 1.0 on ACT tiles,
    # Schraudolph-of-zero on DVE tiles; engine known from the tile pattern)
    for bt in range(B_TILES):
        t = (n_chunks - 1) * B_TILES + bt
        pad_val = PAD_EXP_DVE if _is_dve(t) else 1.0
        sumexp[bt * P : (bt + 1) * P] -= N_CORES * n_pad * pad_val

    target = np.einsum(
        "bd,bd->b", xn.astype(np.float64), W[labels].astype(np.float64)
    )
    tgt = np.clip(target, -1.0 + EPS, 1.0 - EPS)
    numerator = S_SCALE * np.cos(np.arccos(tgt) + MARGIN)
    excl = sumexp - np.exp(S_SCALE * tgt)
    L = numerator - np.log(np.exp(numerator) + excl)
    return np.array(-L.mean(), dtype=np.float32)
